# revision 1
# baseline (speedup 1.0000x reference)
"""DeepClusteringLoss Trainium2 kernel.

loss = (||V^T V||_F^2 - 2 ||V^T E||_F^2 + ||E^T E||_F^2) / (B*N)
summed over batch, with E = embeddings.reshape(B, N, D), V =
assignments.reshape(B, N, S), N = F*T.

Sharding: data-parallel over batch; one core per batch element; the host
sums the 8 per-core partials (the scalar "all-reduce") and divides by
B*N.

Per-core pipeline (measured ~73-76 us HW exec on clean runs; an
environmental SDMA-engine-15 slowdown intermittently adds ~9 us):
- GLOBAL partition map: partition p owns rows [p*1024, (p+1)*1024).
  Chunk c = column c of every partition = 128 rows.
- All of V (2 MB) is cast-DMAed (fp32 HBM -> fp16 SBUF, SWDGE) up front
  into a resident tile; E streams as 14 column-slice DMAs into resident
  tiles (slice 0 via HWDGE in fp32 so the SDMA engines get work before
  the SWDGE Q7 path boots; the rest SWDGE fp16 cast-DMAs) (no buffer reuse -> every DMA enqueues immediately,
  per-partition contiguous segments up to 20 KB, minimal descriptor
  overhead, ~96% SDMA occupancy).  num_swdge_queues=2.
- Interleave copies (DVE for E, ACT for V) build chunk-PAIR operands
  [V_2q | E_2q | pad20 | V_2q+1 | E_2q+1] (128 x 108 fp16): ONE matmul
  per two chunks -> 514 PE instructions instead of 2050 (the PE stream
  is 90% of instruction-fetch bytes, which ride DMA engine 0, the
  completion straggler).  Even/odd Grams accumulate at PSUM partition
  bases 0/64; pad and cross-term cells are never read.
- Epilogue just dumps the raw 108x108 PSUM accumulator; the host adds
  the even/odd diagonal blocks and reduces to the scalar partial in
  float64 (exact).
"""

import os
from contextlib import ExitStack

import numpy as np

import concourse.bacc as bacc
import concourse.mybir as mybir
import concourse.tile as tile
from concourse.bass_utils import run_bass_kernel_spmd

B, F, T, D, S = 8, 256, 512, 40, 4
N = F * T              # rows per core (131072)
SD = S + D             # 44 combined features
PW = 108               # paired-chunk width: 44 | 20 pad | 44
P = 128                # partitions
U = N // P             # rows per partition in the global map (1024)
N_CORES = 8

MM_DT_NAME = os.environ.get("KERNEL_MM_DT", "float16")
SWDGE_Q = int(os.environ.get("KERNEL_SWDGE_Q", "2"))
PAIRED = os.environ.get("KERNEL_PAIRED", "1") == "1"

# chunks per E-slice (all even).  First slice modest so the PE pipeline
# starts early; the tail tapers so the last transfer is tiny.
SLICES = [32, 32] + [128] * 6 + [64, 48, 32, 24, 16, 8]
assert sum(SLICES) == U
assert all(ub % 2 == 0 for ub in SLICES)

_nc_cache = {}


def _build_nc(key):
    (mm_dt_name, paired) = key
    mm_dt = getattr(mybir.dt, mm_dt_name)
    f32 = mybir.dt.float32

    nc = bacc.Bacc("TRN2", target_bir_lowering=False, debug=False,
                   num_swdge_queues=SWDGE_Q)
    E = nc.dram_tensor("embeddings", (N, D), f32, kind="ExternalInput")
    V = nc.dram_tensor("assignments", (N, S), f32, kind="ExternalInput")
    OUT = nc.dram_tensor("partial", (PW, PW), f32, kind="ExternalOutput")

    # global-map DRAM views: partition p <- rows [p*U, (p+1)*U)
    e_g = E[:, :].rearrange("(p u) d -> p (u d)", p=P)   # [128, U*D]
    v_g = V[:, :].rearrange("(p u) s -> p (u s)", p=P)   # [128, U*S]

    with tile.TileContext(nc) as tc, ExitStack() as ctx:
        res_pool = ctx.enter_context(tc.tile_pool(name="res", bufs=1))
        w_pool = ctx.enter_context(tc.tile_pool(name="w", bufs=3))
        psum_pool = ctx.enter_context(tc.tile_pool(name="ps", bufs=1, space="PSUM"))
        gw = PW if paired else SD
        g_ps = psum_pool.tile([gw, gw], f32, tag="g")

        # V up front: one 2 MB cast-DMA into a resident fp16 tile.
        v_all = res_pool.tile([P, U * S], mm_dt, tag="v")
        nc.gpsimd.dma_start(out=v_all[:], in_=v_g)
        v3 = v_all[:].rearrange("p (u s) -> p u s", s=S)

        # E slices: resident fp16 tiles, one cast-DMA each.
        # Slice 0 goes through HWDGE in fp32: the SP sequencer issues it
        # ~1.5us before the SWDGE Q7 path boots, so the SDMA engines have
        # work earlier; its fp32->fp16 cast happens in the interleave
        # copies.  The rest are SWDGE cast-DMAs writing fp16.
        e_tiles = []
        c0 = 0
        for k, ub in enumerate(SLICES):
            if k == 0:
                e_t = res_pool.tile([P, ub * D], f32, tag=f"e{k}")
                nc.sync.dma_start(
                    out=e_t[:], in_=e_g[:, c0 * D:(c0 + ub) * D])
            else:
                e_t = res_pool.tile([P, ub * D], mm_dt, tag=f"e{k}")
                nc.gpsimd.dma_start(
                    out=e_t[:], in_=e_g[:, c0 * D:(c0 + ub) * D])
            e_tiles.append((e_t, c0, ub))
            c0 += ub

        pair = 0
        chunk = 0
        n_pairs = N // (2 * P)
        for k, (e_t, c0, ub) in enumerate(e_tiles):
            last = k == len(e_tiles) - 1
            if paired:
                nq = ub // 2
                w_t = w_pool.tile([P, nq * PW], mm_dt, tag="w")
                w4 = w_t[:].rearrange("p (q c) -> p q c", c=PW)
                e2 = e_t[:].rearrange("p (q r) -> p q r", r=2 * D)
                v2 = v_all[:, c0 * S:(c0 + ub) * S].rearrange(
                    "p (q r) -> p q r", r=2 * S)
                nc.vector.tensor_copy(w4[:, :, S:SD], e2[:, :, 0:D])
                nc.vector.tensor_copy(w4[:, :, 64 + S:64 + SD], e2[:, :, D:2 * D])
                nc.scalar.copy(w4[:, :, 0:S], v2[:, :, 0:S])
                nc.scalar.copy(w4[:, :, 64:64 + S], v2[:, :, S:2 * S])
                for q in range(nq):
                    wq = w_t[:, q * PW:(q + 1) * PW]
                    nc.tensor.matmul(
                        g_ps[:], wq, wq,
                        start=(pair == 0),
                        stop=(last and q == nq - 1),
                    )
                    pair += 1
            else:
                w_t = w_pool.tile([P, ub * SD], mm_dt, tag="w")
                w3 = w_t[:].rearrange("p (u c) -> p u c", c=SD)
                nc.vector.tensor_copy(
                    w3[:, :, S:SD], e_t[:].rearrange("p (u d) -> p u d", d=D))
                nc.scalar.copy(w3[:, :, 0:S], v3[:, c0:c0 + ub, :])
                for u in range(ub):
                    wu = w_t[:, u * SD:(u + 1) * SD]
                    nc.tensor.matmul(
                        g_ps[:], wu, wu,
                        start=(chunk == 0),
                        stop=(last and u == ub - 1),
                    )
                    chunk += 1

        # Epilogue: dump only the two 44x44 diagonal Gram blocks of the
        # PSUM accumulator, each on its own HWDGE ring (SP and ACT) so the
        # descriptor generation for the two OUT transfers runs in
        # parallel; the host adds the blocks and reduces to the scalar
        # partial (exact, in float64) alongside the cross-core sum.
        ep = ctx.enter_context(tc.tile_pool(name="ep", bufs=1))
        if paired:
            ge_sb = ep.tile([SD, SD], f32, tag="ge")
            go_sb = ep.tile([64 + SD, SD], f32, tag="go")
            nc.vector.tensor_copy(ge_sb[:], g_ps[0:SD, 0:SD])
            nc.vector.tensor_copy(
                go_sb[64:64 + SD, :], g_ps[64:64 + SD, 64:64 + SD])
            nc.sync.dma_start(out=OUT[0:SD, 0:SD], in_=ge_sb[:])
            nc.sync.dma_start(
                out=OUT[64:64 + SD, 64:64 + SD], in_=go_sb[64:64 + SD, :])
        else:
            g_sb = ep.tile([gw, gw], f32, tag="gsb")
            nc.vector.tensor_copy(g_sb[:], g_ps[:])
            nc.sync.dma_start(out=OUT[0:gw, 0:gw], in_=g_sb[:])

    nc.finalize()
    return nc


def _get_nc():
    key = (MM_DT_NAME, PAIRED)
    if key not in _nc_cache:
        _nc_cache[key] = _build_nc(key)
    return _nc_cache[key]


def _run(embeddings: np.ndarray, assignments: np.ndarray, trace: bool = False):
    nc = _get_nc()
    in_maps = []
    for i in range(N_CORES):
        in_maps.append({
            "embeddings": np.ascontiguousarray(
                embeddings[i].reshape(N, D).astype(np.float32, copy=False)),
            "assignments": np.ascontiguousarray(
                assignments[i].reshape(N, S).astype(np.float32, copy=False)),
        })
    try:
        res = run_bass_kernel_spmd(
            nc, in_maps, core_ids=list(range(N_CORES)), trace=trace
        )
    except Exception:
        res = run_bass_kernel_spmd(
            nc, in_maps, core_ids=list(range(N_CORES)), trace=trace
        )
    partials = []
    for r in res.results:
        gp = np.asarray(r["partial"], dtype=np.float64)
        if PAIRED:
            G = gp[0:SD, 0:SD] + gp[64:64 + SD, 64:64 + SD]
        else:
            G = gp[0:SD, 0:SD]
        bm = G[0:S, S:SD]
        partials.append(np.sum(G * G) - 4.0 * np.sum(bm * bm))
    total = np.float32(np.sum(np.asarray(partials, dtype=np.float64)) / (B * N))
    return np.asarray(total, dtype=np.float32), res


def kernel(embeddings: np.ndarray, assignments: np.ndarray) -> np.ndarray:
    out, _ = _run(embeddings, assignments, trace=False)
    return out



# revision 2
# speedup vs baseline: 1.0443x; 1.0443x over previous
"""DeepClusteringLoss Trainium2 kernel.

loss = (||V^T V||_F^2 - 2 ||V^T E||_F^2 + ||E^T E||_F^2) / (B*N)
summed over batch, with E = embeddings.reshape(B, N, D), V =
assignments.reshape(B, N, S), N = F*T.

Sharding: data-parallel over batch; one core per batch element; the host
sums the 8 per-core partials (the scalar "all-reduce") and divides by
B*N.

Per-core pipeline (DMA/HBM-bound: 23.07 MB fp32 input @ ~358 GB/s/core
=> ~64.4 us transfer floor):
- GLOBAL partition map: partition p owns rows [p*1024, (p+1)*1024).
  Chunk c = column c of every partition = 128 rows.
- ALL streaming is HWDGE (SP + ACT rings) in fp32: HWDGE descriptor
  generation is RTL (no Q7 SWDGE boot delay, which cost ~3-6 us of
  16-engine idle at the front), every DMA sprays all 16 SDMA engines
  evenly (the SWDGE baseline left engines 11-15 ~6 us underloaded), and
  since HBM (~358 GB/s) binds before the SBUF AXI fabric (435 GB/s),
  streaming fp32 instead of cast-to-fp16 costs no bandwidth.
- V (2 MB) goes first on the ACT ring into a resident fp32 tile; E
  streams as 19 tapered column-slices (14x64 + 48,32,24,16,8 chunks)
  alternating SP/ACT rings through an 8-deep fp32 ring buffer.
- Interleave copies (DVE for E, ACT for V) cast fp32->fp16 while
  building chunk-PAIR operands [V_2q | E_2q | pad20 | V_2q+1 | E_2q+1]
  (128 x 108 fp16): ONE matmul per two chunks -> 512 PE instruction
  pairs.  Even/odd Grams accumulate at PSUM partition bases 0/64; pad
  and cross-term cells are never read.
- Epilogue dumps the two 44x44 diagonal Gram blocks (SP + ACT rings in
  parallel); the host adds them and reduces to the scalar partial in
  float64 (exact).
"""

import os
from contextlib import ExitStack

import numpy as np

import concourse.bacc as bacc
import concourse.mybir as mybir
import concourse.tile as tile
from concourse.bass_utils import run_bass_kernel_spmd

B, F, T, D, S = 8, 256, 512, 40, 4
N = F * T              # rows per core (131072)
SD = S + D             # 44 combined features
PW = 108               # paired-chunk width: 44 | 20 pad | 44
P = 128                # partitions
U = N // P             # rows per partition in the global map (1024)
N_CORES = 8

MM_DT_NAME = os.environ.get("KERNEL_MM_DT", "float16")
RING = os.environ.get("KERNEL_RING", "alt")   # "alt" | "sp"
EBUFS = int(os.environ.get("KERNEL_EBUFS", "8"))
WBUFS = int(os.environ.get("KERNEL_WBUFS", "4"))

# E slice taper: big uniform slices for line-rate DMA, small tail so the
# last-slice copy+matmul+epilogue dependency chain is short.
SLICES = [64] * 14 + [48, 32, 24, 16, 8]
assert sum(SLICES) == U
assert all(ub % 2 == 0 for ub in SLICES)

_nc_cache = {}


def _build_nc(key):
    (mm_dt_name, ring_mode, ebufs, wbufs) = key
    mm_dt = getattr(mybir.dt, mm_dt_name)
    f32 = mybir.dt.float32

    nc = bacc.Bacc("TRN2", target_bir_lowering=False, debug=False)
    E = nc.dram_tensor("embeddings", (N, D), f32, kind="ExternalInput")
    V = nc.dram_tensor("assignments", (N, S), f32, kind="ExternalInput")
    OUT = nc.dram_tensor("partial", (PW, PW), f32, kind="ExternalOutput")

    # global-map DRAM views: partition p <- rows [p*U, (p+1)*U)
    e_g = E[:, :].rearrange("(p u) d -> p (u d)", p=P)   # [128, U*D]
    v_g = V[:, :].rearrange("(p u) s -> p (u s)", p=P)   # [128, U*S]

    with tile.TileContext(nc) as tc, ExitStack() as ctx:
        res_pool = ctx.enter_context(tc.tile_pool(name="res", bufs=1))
        e_pool = ctx.enter_context(tc.tile_pool(name="e", bufs=ebufs))
        w_pool = ctx.enter_context(tc.tile_pool(name="w", bufs=wbufs))
        psum_pool = ctx.enter_context(tc.tile_pool(name="ps", bufs=1, space="PSUM"))
        g_ps = psum_pool.tile([PW, PW], f32, tag="g")

        # V up front: one 2 MB fp32 HWDGE DMA on the ACT ring into a
        # resident tile; the ACT interleave copies cast it later.
        v_all = res_pool.tile([P, U * S], f32, tag="v")
        nc.scalar.dma_start(out=v_all[:], in_=v_g)

        pair = 0
        n_pairs = U // 2
        c0 = 0
        for k, ub in enumerate(SLICES):
            last = k == len(SLICES) - 1
            # E slice: fp32 HWDGE DMA into one of `ebufs` ring slots.
            e_t = e_pool.tile([P, ub * D], f32, tag="e")
            eng = nc.sync if (ring_mode == "sp" or k % 2 == 0) else nc.scalar
            eng.dma_start(out=e_t[:], in_=e_g[:, c0 * D:(c0 + ub) * D])

            nq = ub // 2
            w_t = w_pool.tile([P, nq * PW], mm_dt, tag="w")
            w4 = w_t[:].rearrange("p (q c) -> p q c", c=PW)
            e2 = e_t[:].rearrange("p (q r) -> p q r", r=2 * D)
            v2 = v_all[:, c0 * S:(c0 + ub) * S].rearrange(
                "p (q r) -> p q r", r=2 * S)
            nc.vector.tensor_copy(w4[:, :, S:SD], e2[:, :, 0:D])
            nc.vector.tensor_copy(w4[:, :, 64 + S:64 + SD], e2[:, :, D:2 * D])
            nc.scalar.copy(w4[:, :, 0:S], v2[:, :, 0:S])
            nc.scalar.copy(w4[:, :, 64:64 + S], v2[:, :, S:2 * S])
            for q in range(nq):
                wq = w_t[:, q * PW:(q + 1) * PW]
                nc.tensor.matmul(
                    g_ps[:], wq, wq,
                    start=(pair == 0),
                    stop=(last and q == nq - 1),
                )
                pair += 1
            c0 += ub

        # Epilogue: dump only the two 44x44 diagonal Gram blocks of the
        # PSUM accumulator, each on its own HWDGE ring (SP and ACT) so
        # the descriptor generation for the two OUT transfers runs in
        # parallel; the host adds the blocks and reduces to the scalar
        # partial (exact, in float64) alongside the cross-core sum.
        ep = ctx.enter_context(tc.tile_pool(name="ep", bufs=1))
        ge_sb = ep.tile([SD, SD], f32, tag="ge")
        go_sb = ep.tile([64 + SD, SD], f32, tag="go")
        nc.vector.tensor_copy(ge_sb[:], g_ps[0:SD, 0:SD])
        nc.vector.tensor_copy(
            go_sb[64:64 + SD, :], g_ps[64:64 + SD, 64:64 + SD])
        nc.sync.dma_start(out=OUT[0:SD, 0:SD], in_=ge_sb[:])
        nc.scalar.dma_start(
            out=OUT[64:64 + SD, 64:64 + SD], in_=go_sb[64:64 + SD, :])

    nc.finalize()
    return nc


def _get_nc():
    key = (MM_DT_NAME, RING, EBUFS, WBUFS)
    if key not in _nc_cache:
        _nc_cache[key] = _build_nc(key)
    return _nc_cache[key]


def _run(embeddings: np.ndarray, assignments: np.ndarray, trace: bool = False):
    nc = _get_nc()
    in_maps = []
    for i in range(N_CORES):
        in_maps.append({
            "embeddings": np.ascontiguousarray(
                embeddings[i].reshape(N, D).astype(np.float32, copy=False)),
            "assignments": np.ascontiguousarray(
                assignments[i].reshape(N, S).astype(np.float32, copy=False)),
        })
    try:
        res = run_bass_kernel_spmd(
            nc, in_maps, core_ids=list(range(N_CORES)), trace=trace
        )
    except Exception:
        res = run_bass_kernel_spmd(
            nc, in_maps, core_ids=list(range(N_CORES)), trace=trace
        )
    partials = []
    for r in res.results:
        gp = np.asarray(r["partial"], dtype=np.float64)
        G = gp[0:SD, 0:SD] + gp[64:64 + SD, 64:64 + SD]
        bm = G[0:S, S:SD]
        partials.append(np.sum(G * G) - 4.0 * np.sum(bm * bm))
    total = np.float32(np.sum(np.asarray(partials, dtype=np.float64)) / (B * N))
    return np.asarray(total, dtype=np.float32), res


def kernel(embeddings: np.ndarray, assignments: np.ndarray) -> np.ndarray:
    out, _ = _run(embeddings, assignments, trace=False)
    return out
